# revision 24
# baseline (speedup 1.0000x reference)
"""Trainium2 Bass kernel for nn_BatchDelayProcessor.

Computes, per batch row (B=64, T=441000, D=22050 delay, 20 blocks):
    delayed[t] = 0                          , t < D
    delayed[t] = x[t-D] + 0.3*delayed[t-D]  , t >= D
    out[t]     = 0.5*x[t] + 0.5*delayed[t]

Unrolling the block recurrence, out_p = sum_j W[p,j] * x_j with the banded
lower-triangular W[p,p] = 0.5, W[p,j] = 0.5*0.3^(p-1-j) (j<p) -- i.e. a
20x20 matmul over the block axis, identical for every row.  So: lay out
SBUF as partition = (row, block), free = sample offset, and let the PE do
the whole recurrence as OUT = W @ X with a block-diagonal stationary
(4 rows/group -> 80x80), bf16 in / f32 PSUM out.

Why this layout wins: DMA descriptors become one long contiguous DRAM run
per (row, block, column-slab) -- 29 KB reads instead of the 5.9 KB of
the partition=(row, 1/15th-block) STT formulation whose HWDGE descriptor
generation (~64 ns/desc, 2400 descs) capped the kernel at ~118 us.  All
x/y DMA goes through SWDGE (GpSimd): its emission is ~0.7 us per
dma_start regardless of descriptor count, and queue 0 spreads over all
16 SDMA engines (~375 GB/s) where the two HWDGE rings share only 8.
The f32->bf16 input cast happens inside the load DMA (SWDGE-only
feature), so no engine pass is spent on it.

Per core: 2 row-groups x 3 column-slabs of 7350 samples, fully buffered
in SBUF (x: 88 KB/partition bf16, out: 88 KB) so loads are emitted
back-to-back with NO waits and stores never gate compute.  15 matmuls of
490 columns per slab (PSUM bank per matmul, 8 banks round-robin).  PSUM
f32 -> SBUF bf16 copies split DVE (even matmul idx) / ACT (odd idx) --
PSUM reads are 1 elem/lane/cycle, one engine alone would bottleneck.
y is written as bf16 (halves store HBM traffic; rel-err budget 2e-2 vs
bf16's ~2e-3) and upcast to f32 on the host.

Engine split:
  GpSimd: x loads then y stores via SWDGE queue 0
  PE:     90 bf16 matmuls (80-partition block-diag W)
  DVE:    PSUM->SBUF bf16 copies, even matmul indices
  ACT:    PSUM->SBUF bf16 copies, odd matmul indices
  SP:     W load (HWDGE; overlaps the ~8.5us GpSimd engine preamble)
"""

from contextlib import ExitStack

import numpy as np

import concourse.bass as bass
import concourse.mybir as mybir
from concourse.bass_utils import run_bass_kernel_spmd

B, T = 64, 441000
D, NBLK = 22050, 20
NCORES = 8
ROWS = B // NCORES          # 8 rows per core
GROUPS = 2                  # row groups per core
GR = ROWS // GROUPS         # 4 rows per group
P = GR * NBLK               # 80 partitions: (row-in-group, block)
MMCOL = 490                 # columns per matmul (<=512 psum bank cap)
# Per-group column slabs (c0, c1), sized so matmul consumption tracks
# load arrival without stalls.
SLABS = [(0, 7350), (7350, 14700), (14700, 22050)]
NSLAB = len(SLABS)
NBANK = 8                   # PSUM banks in round-robin

F32 = mybir.dt.float32
BF16 = mybir.dt.bfloat16

# Global slab schedule: alternate groups for an even pipeline.
SLAB_ORDER = [(t % 2, t // 2) for t in range(GROUPS * NSLAB)]
NT = len(SLAB_ORDER)        # 6
CHUNKS = [(c1 - c0) // MMCOL for c0, c1 in SLABS]  # [20, 20, 5]
NMM = GROUPS * sum(CHUNKS)  # 90
# global matmul index at the start of each scheduled slab
MM_BASE = []
_acc = 0
for _t in range(NT):
    MM_BASE.append(_acc)
    _acc += CHUNKS[SLAB_ORDER[_t][1]]


def _weights() -> np.ndarray:
    """lhsT for nc.tensor.matmul: out = lhsT.T @ rhs.

    lhsT[(r,j), (r',p)] = W[p, j] if r == r' else 0, with
    W[p, j] = 0.5*(p==j) + 0.5*0.3^(p-1-j)*(j<p).
    """
    W = np.zeros((NBLK, NBLK), np.float64)
    for p in range(NBLK):
        W[p, p] = 0.5
        for j in range(p):
            W[p, j] = 0.5 * 0.3 ** (p - 1 - j)
    import ml_dtypes

    return np.kron(np.eye(GR), W.T).astype(ml_dtypes.bfloat16)


def build_nc() -> bass.Bass:
    nc = bass.Bass(trn_type="TRN2")
    x = nc.declare_dram_parameter("x", [ROWS, T], F32, isOutput=False)
    w = nc.declare_dram_parameter("w", [P, P], BF16, isOutput=False)
    y = nc.declare_dram_parameter("y", [ROWS, T], BF16, isOutput=True)
    xv = x.rearrange("r (j c) -> r j c", j=NBLK)   # (8, 20, 22050)
    yv = y.rearrange("r (j c) -> r j c", j=NBLK)

    with ExitStack() as ctx:
        block = ctx.enter_context(nc.Block())
        wbuf = ctx.enter_context(nc.sbuf_tensor("wbuf", [P, P], BF16))
        # Full group resident: slab s of group g lives at columns SLABS[s]
        xbuf = [
            ctx.enter_context(
                nc.sbuf_tensor(f"xbuf{g}", [P, D], BF16)
            )
            for g in range(GROUPS)
        ]
        obuf = [
            ctx.enter_context(
                nc.sbuf_tensor(f"obuf{g}", [P, D], BF16)
            )
            for g in range(GROUPS)
        ]
        psum = [
            ctx.enter_context(nc.psum_tensor(f"ps{b}", [P, MMCOL], F32))
            for b in range(NBANK)
        ]
        s_w = ctx.enter_context(nc.semaphore("s_w"))
        s_x = [
            [
                ctx.enter_context(nc.semaphore(f"s_x{g}_{s}"))
                for s in range(NSLAB)
            ]
            for g in range(GROUPS)
        ]
        # per-piece load sems: loads stream in PIECE-column chunks and the
        # PE consumes at nearly the delivery rate, so piece-granular sync
        # lets matmuls start ~4us earlier (one sem per piece -- a shared
        # sem could reach 16 from two in-flight pieces' partial incs)
        s_xp = [
            [
                [
                    ctx.enter_context(nc.semaphore(f"s_xp{g}_{s}_{p}"))
                    for p in range(3)
                ]
                for s in range(NSLAB)
            ]
            for g in range(GROUPS)
        ]
        s_mm = ctx.enter_context(nc.semaphore("s_mm"))
        s_cpd = ctx.enter_context(nc.semaphore("s_cpd"))
        s_cpa = ctx.enter_context(nc.semaphore("s_cpa"))

        def xslab(g, s):
            return xbuf[g][:, SLABS[s][0] : SLABS[s][1]]

        def oslab(g, s):
            return obuf[g][:, SLABS[s][0] : SLABS[s][1]]

        # copies done counts: copy idx -> (# s_cpd incs, # s_cpa incs) after it
        def copies_done(last_idx):
            return (last_idx + 2) // 2, (last_idx + 1) // 2

        def load(gp, g, c0, c1, sem):
            gp.dma_start(
                out=xbuf[g][:, c0:c1],
                in_=xv[g * GR : (g + 1) * GR, :, c0:c1],
            ).then_inc(sem, 16)

        @block.sync
        def _(sp):
            # W rides the otherwise-idle SP HWDGE ring, off the q0 path.
            sp.dma_start(out=wbuf[:, :], in_=w[:, :]).then_inc(s_w, 16)

        PIECE = 5 * MMCOL           # 2450 columns per load piece

        @block.gpsimd
        def _(gp):
            # All loads up front, zero waits, in consumption order and
            # PIECE-sized chunks: the whole group ends up resident.
            for g, s in SLAB_ORDER:
                for p in range(3):
                    c0 = SLABS[s][0] + p * PIECE
                    load(gp, g, c0, c0 + PIECE, s_xp[g][s][p])
            # Stores drain as each slab's copies retire, in half-slab
            # pieces so the SDMA backlog starts draining earlier; nothing
            # waits on them (no obuf reuse), the Block-exit drain ensures
            # completion.  then_inc only because DGE requires sync info.
            for t, (g, s) in enumerate(SLAB_ORDER):
                h = (CHUNKS[s] + 1) // 2
                cm = SLABS[s][0] + h * MMCOL
                nd, na = copies_done(MM_BASE[t] + h - 1)
                gp.wait_ge(s_cpd, nd)
                gp.wait_ge(s_cpa, na)
                gp.dma_start(
                    out=yv[g * GR : (g + 1) * GR, :, SLABS[s][0] : cm],
                    in_=obuf[g][:, SLABS[s][0] : cm],
                ).then_inc(s_x[g][s], 16)
                nd, na = copies_done(MM_BASE[t] + CHUNKS[s] - 1)
                gp.wait_ge(s_cpd, nd)
                gp.wait_ge(s_cpa, na)
                gp.dma_start(
                    out=yv[g * GR : (g + 1) * GR, :, cm : SLABS[s][1]],
                    in_=obuf[g][:, cm : SLABS[s][1]],
                ).then_inc(s_x[g][s], 16)

        @block.tensor
        def _(te):
            te.wait_ge(s_w, 16)
            for t, (g, s) in enumerate(SLAB_ORDER):
                for i in range(CHUNKS[s]):
                    idx = MM_BASE[t] + i
                    if i % 5 == 0:
                        te.wait_ge(s_xp[g][s][i // 5], 16)
                    if idx >= NBANK:
                        # PSUM bank WAR: copy idx-NBANK retired
                        old = idx - NBANK
                        if old % 2 == 0:
                            te.wait_ge(s_cpd, old // 2 + 1)
                        else:
                            te.wait_ge(s_cpa, old // 2 + 1)
                    c0 = SLABS[s][0] + i * MMCOL
                    nc.tensor.matmul(
                        out=psum[idx % NBANK][:, :],
                        lhsT=wbuf[:, :],
                        rhs=xbuf[g][:, c0 : c0 + MMCOL],
                        start=True,
                        stop=True,
                    ).then_inc(s_mm, 1)

        def _copy_prog(eng, vec, parity, sem):
            for t, (g, s) in enumerate(SLAB_ORDER):
                for i in range(CHUNKS[s]):
                    idx = MM_BASE[t] + i
                    if idx % 2 != parity:
                        continue
                    eng.wait_ge(s_mm, idx + 1)
                    c0 = SLABS[s][0] + i * MMCOL
                    vec(
                        obuf[g][:, c0 : c0 + MMCOL],
                        psum[idx % NBANK][:, :],
                    ).then_inc(sem, 1)

        @block.vector
        def _(ve):
            _copy_prog(ve, nc.vector.tensor_copy, 0, s_cpd)

        @block.scalar
        def _(sc):
            _copy_prog(sc, nc.scalar.copy, 1, s_cpa)

    return nc


_NC_CACHE = None


def _get_nc() -> bass.Bass:
    global _NC_CACHE
    if _NC_CACHE is None:
        _NC_CACHE = build_nc()
    return _NC_CACHE


_W = _weights()


def _shard(x: np.ndarray) -> list[dict[str, np.ndarray]]:
    x = np.ascontiguousarray(np.asarray(x, dtype=np.float32))
    assert x.shape == (B, T), x.shape
    return [
        {
            "x": np.ascontiguousarray(x[i * ROWS : (i + 1) * ROWS]),
            "w": _W,
        }
        for i in range(NCORES)
    ]


def kernel(x: np.ndarray) -> np.ndarray:
    nc = _get_nc()
    res = run_bass_kernel_spmd(nc, _shard(x), core_ids=list(range(NCORES)))
    return np.concatenate(
        [np.asarray(r["y"]) for r in res.results], axis=0
    ).astype(np.float32)


def kernel_profiled(x: np.ndarray):
    """Like kernel() but with NTFF tracing; returns (out, BassKernelResults)."""
    nc = _get_nc()
    res = run_bass_kernel_spmd(
        nc, _shard(x), core_ids=list(range(NCORES)), trace=True
    )
    out = np.concatenate(
        [np.asarray(r["y"]) for r in res.results], axis=0
    ).astype(np.float32)
    return out, res


# revision 26
# speedup vs baseline: 1.0336x; 1.0336x over previous
"""Trainium2 Bass kernel for nn_BatchDelayProcessor.

Computes, per batch row (B=64, T=441000, D=22050 delay, 20 blocks):
    delayed[t] = 0                          , t < D
    delayed[t] = x[t-D] + 0.3*delayed[t-D]  , t >= D
    out[t]     = 0.5*x[t] + 0.5*delayed[t]

Unrolling the block recurrence, out_p = sum_j W[p,j] * x_j with the banded
lower-triangular W[p,p] = 0.5, W[p,j] = 0.5*0.3^(p-1-j) (j<p) -- i.e. a
20x20 matmul over the block axis, identical for every row.  So: lay out
SBUF as partition = (row, block), free = sample offset, and let the PE do
the whole recurrence as OUT = W @ X with a block-diagonal stationary
(4 rows/group -> 80x80), bf16 in / f32 PSUM out.

Why this layout wins: DMA descriptors become one long contiguous DRAM run
per (row, block, column-slab) -- 29 KB reads instead of the 5.9 KB of
the partition=(row, 1/15th-block) STT formulation whose HWDGE descriptor
generation (~64 ns/desc, 2400 descs) capped the kernel at ~118 us.  All
x/y DMA goes through SWDGE (GpSimd): its emission is ~0.7 us per
dma_start regardless of descriptor count, and queue 0 spreads over all
16 SDMA engines (~375 GB/s) where the two HWDGE rings share only 8.
The f32->bf16 input cast happens inside the load DMA (SWDGE-only
feature), so no engine pass is spent on it.

Per core: 2 row-groups x 3 column-slabs of 7350 samples, fully buffered
in SBUF (x: 88 KB/partition bf16, out: 88 KB) so loads are emitted
back-to-back with NO waits and stores never gate compute.  15 matmuls of
490 columns per slab (PSUM bank per matmul, 8 banks round-robin).  PSUM
f32 -> SBUF bf16 copies split DVE (even matmul idx) / ACT (odd idx) --
PSUM reads are 1 elem/lane/cycle, one engine alone would bottleneck.
y is written as bf16 (halves store HBM traffic; rel-err budget 2e-2 vs
bf16's ~2e-3) and upcast to f32 on the host.

Engine split:
  GpSimd: x loads then y stores via SWDGE queue 0
  PE:     90 bf16 matmuls (80-partition block-diag W)
  DVE:    PSUM->SBUF bf16 copies, even matmul indices
  ACT:    PSUM->SBUF bf16 copies, odd matmul indices
  SP:     W load (HWDGE; overlaps the ~8.5us GpSimd engine preamble)
"""

from contextlib import ExitStack

import numpy as np

import concourse.bass as bass
import concourse.mybir as mybir
from concourse.bass_utils import run_bass_kernel_spmd

B, T = 64, 441000
D, NBLK = 22050, 20
NCORES = 8
ROWS = B // NCORES          # 8 rows per core
GROUPS = 2                  # row groups per core
GR = ROWS // GROUPS         # 4 rows per group
P = GR * NBLK               # 80 partitions: (row-in-group, block)
MMCOL = 490                 # columns per matmul (<=512 psum bank cap)
# Per-group column slabs (c0, c1), sized so matmul consumption tracks
# load arrival without stalls.
SLABS = [(0, 7350), (7350, 14700), (14700, 22050)]
NSLAB = len(SLABS)
NBANK = 8                   # PSUM banks in round-robin

F32 = mybir.dt.float32
BF16 = mybir.dt.bfloat16

# Global slab schedule: alternate groups for an even pipeline.
SLAB_ORDER = [(t % 2, t // 2) for t in range(GROUPS * NSLAB)]
NT = len(SLAB_ORDER)        # 6
CHUNKS = [(c1 - c0) // MMCOL for c0, c1 in SLABS]  # [20, 20, 5]
NMM = GROUPS * sum(CHUNKS)  # 90
# global matmul index at the start of each scheduled slab
MM_BASE = []
_acc = 0
for _t in range(NT):
    MM_BASE.append(_acc)
    _acc += CHUNKS[SLAB_ORDER[_t][1]]


def _weights() -> np.ndarray:
    """lhsT for nc.tensor.matmul: out = lhsT.T @ rhs.

    lhsT[(r,j), (r',p)] = W[p, j] if r == r' else 0, with
    W[p, j] = 0.5*(p==j) + 0.5*0.3^(p-1-j)*(j<p).
    """
    W = np.zeros((NBLK, NBLK), np.float64)
    for p in range(NBLK):
        W[p, p] = 0.5
        for j in range(p):
            W[p, j] = 0.5 * 0.3 ** (p - 1 - j)
    import ml_dtypes

    return np.kron(np.eye(GR), W.T).astype(ml_dtypes.bfloat16)


def build_nc() -> bass.Bass:
    nc = bass.Bass(trn_type="TRN2")
    x = nc.declare_dram_parameter("x", [ROWS, T], F32, isOutput=False)
    w = nc.declare_dram_parameter("w", [P, P], BF16, isOutput=False)
    y = nc.declare_dram_parameter("y", [ROWS, T], BF16, isOutput=True)
    xv = x.rearrange("r (j c) -> r j c", j=NBLK)   # (8, 20, 22050)
    yv = y.rearrange("r (j c) -> r j c", j=NBLK)

    with ExitStack() as ctx:
        block = ctx.enter_context(nc.Block())
        wbuf = ctx.enter_context(nc.sbuf_tensor("wbuf", [P, P], BF16))
        # Full group resident: slab s of group g lives at columns SLABS[s]
        xbuf = [
            ctx.enter_context(
                nc.sbuf_tensor(f"xbuf{g}", [P, D], BF16)
            )
            for g in range(GROUPS)
        ]
        obuf = [
            ctx.enter_context(
                nc.sbuf_tensor(f"obuf{g}", [P, D], BF16)
            )
            for g in range(GROUPS)
        ]
        psum = [
            ctx.enter_context(nc.psum_tensor(f"ps{b}", [P, MMCOL], F32))
            for b in range(NBANK)
        ]
        s_w = ctx.enter_context(nc.semaphore("s_w"))
        s_x = [
            [
                ctx.enter_context(nc.semaphore(f"s_x{g}_{s}"))
                for s in range(NSLAB)
            ]
            for g in range(GROUPS)
        ]
        s_mm = ctx.enter_context(nc.semaphore("s_mm"))
        s_cpd = ctx.enter_context(nc.semaphore("s_cpd"))
        s_cpa = ctx.enter_context(nc.semaphore("s_cpa"))

        def xslab(g, s):
            return xbuf[g][:, SLABS[s][0] : SLABS[s][1]]

        def oslab(g, s):
            return obuf[g][:, SLABS[s][0] : SLABS[s][1]]

        # copies done counts: copy idx -> (# s_cpd incs, # s_cpa incs) after it
        def copies_done(last_idx):
            return (last_idx + 2) // 2, (last_idx + 1) // 2

        def load(gp, g, c0, c1, sem):
            gp.dma_start(
                out=xbuf[g][:, c0:c1],
                in_=xv[g * GR : (g + 1) * GR, :, c0:c1],
            ).then_inc(sem, 16)

        @block.sync
        def _(sp):
            # W rides the otherwise-idle SP HWDGE ring, off the q0 path.
            sp.dma_start(out=wbuf[:, :], in_=w[:, :]).then_inc(s_w, 16)

        @block.gpsimd
        def _(gp):
            # All loads up front, zero waits: the whole group is resident.
            for g, s in SLAB_ORDER:
                load(gp, g, SLABS[s][0], SLABS[s][1], s_x[g][s])
            # Stores drain as each slab's copies retire, in half-slab
            # pieces so the SDMA backlog starts draining earlier; nothing
            # waits on them (no obuf reuse), the Block-exit drain ensures
            # completion.  then_inc only because DGE requires sync info.
            for t, (g, s) in enumerate(SLAB_ORDER):
                bounds = [0, 4, 8, CHUNKS[s]]
                for b in range(len(bounds) - 1):
                    i0, i1 = bounds[b], bounds[b + 1]
                    nd, na = copies_done(MM_BASE[t] + i1 - 1)
                    gp.wait_ge(s_cpd, nd)
                    gp.wait_ge(s_cpa, na)
                    c0 = SLABS[s][0] + i0 * MMCOL
                    c1 = SLABS[s][0] + i1 * MMCOL
                    gp.dma_start(
                        out=yv[g * GR : (g + 1) * GR, :, c0:c1],
                        in_=obuf[g][:, c0:c1],
                    ).then_inc(s_x[g][s], 16)

        @block.tensor
        def _(te):
            te.wait_ge(s_w, 16)
            for t, (g, s) in enumerate(SLAB_ORDER):
                for i in range(CHUNKS[s]):
                    idx = MM_BASE[t] + i
                    if i == 0:
                        te.wait_ge(s_x[g][s], 16)
                    if idx >= NBANK:
                        # PSUM bank WAR: copy idx-NBANK retired
                        old = idx - NBANK
                        if old % 2 == 0:
                            te.wait_ge(s_cpd, old // 2 + 1)
                        else:
                            te.wait_ge(s_cpa, old // 2 + 1)
                    c0 = SLABS[s][0] + i * MMCOL
                    nc.tensor.matmul(
                        out=psum[idx % NBANK][:, :],
                        lhsT=wbuf[:, :],
                        rhs=xbuf[g][:, c0 : c0 + MMCOL],
                        start=True,
                        stop=True,
                    ).then_inc(s_mm, 1)

        def _copy_prog(eng, vec, parity, sem):
            for t, (g, s) in enumerate(SLAB_ORDER):
                for i in range(CHUNKS[s]):
                    idx = MM_BASE[t] + i
                    if idx % 2 != parity:
                        continue
                    eng.wait_ge(s_mm, idx + 1)
                    c0 = SLABS[s][0] + i * MMCOL
                    vec(
                        obuf[g][:, c0 : c0 + MMCOL],
                        psum[idx % NBANK][:, :],
                    ).then_inc(sem, 1)

        @block.vector
        def _(ve):
            _copy_prog(ve, nc.vector.tensor_copy, 0, s_cpd)

        @block.scalar
        def _(sc):
            _copy_prog(sc, nc.scalar.copy, 1, s_cpa)

    return nc


_NC_CACHE = None


def _get_nc() -> bass.Bass:
    global _NC_CACHE
    if _NC_CACHE is None:
        _NC_CACHE = build_nc()
    return _NC_CACHE


_W = _weights()


def _shard(x: np.ndarray) -> list[dict[str, np.ndarray]]:
    x = np.ascontiguousarray(np.asarray(x, dtype=np.float32))
    assert x.shape == (B, T), x.shape
    return [
        {
            "x": np.ascontiguousarray(x[i * ROWS : (i + 1) * ROWS]),
            "w": _W,
        }
        for i in range(NCORES)
    ]


def kernel(x: np.ndarray) -> np.ndarray:
    nc = _get_nc()
    res = run_bass_kernel_spmd(nc, _shard(x), core_ids=list(range(NCORES)))
    return np.concatenate(
        [np.asarray(r["y"]) for r in res.results], axis=0
    ).astype(np.float32)


def kernel_profiled(x: np.ndarray):
    """Like kernel() but with NTFF tracing; returns (out, BassKernelResults)."""
    nc = _get_nc()
    res = run_bass_kernel_spmd(
        nc, _shard(x), core_ids=list(range(NCORES)), trace=True
    )
    out = np.concatenate(
        [np.asarray(r["y"]) for r in res.results], axis=0
    ).astype(np.float32)
    return out, res
